# revision 13
# baseline (speedup 1.0000x reference)
"""MoE (DeepSeek-style naive top-k routing + per-expert SwiGLU) on 8 Trainium2 cores.

Strategy: expert parallelism with host-side token dispatch/combine.
  - Host computes the routing (top_k_index/top_k_weights -> per-expert token
    lists + combine gates), gathers each expert's tokens into a padded
    capacity-C buffer, and hands core e exactly expert e's weights + tokens.
  - Each core runs dense SwiGLU over its C tokens:
        Y^T = W12p^T @ X^T          (GEMM1, contraction over DIM=1024)
        hidden^T = silu(x1) * x2    (pair-interleaved chunk layout)
        out = (hidden^T)^T @ W3     (GEMM2, contraction over H, padded to 768)
    with the per-token combine gate folded into the PSUM->SBUF copy of the
    GEMM2 result.
  - Host scatter-adds the 8 per-expert partial outputs into the [T, DIM] out.

W12 columns are permuted on host into 6 chunks of [x1_p (128) | x2_p (128)]
so that silu(x1)*x2 pairs are partition-aligned in the Y^T layout. H=704 is
zero-padded to 768 (x1 pad cols give silu(0)*0 = 0, harmless for GEMM2).
"""

import os
import sys

for _p in ("/opt/trn_rl_repo",):
    if _p not in sys.path:
        sys.path.insert(0, _p)

import numpy as np

E = 8
DIM = 1024
H = 704
TOPK = 2
HP = 768            # H padded up for W3 row blocks (last block half-valid)
KD = DIM // 128     # contraction tiles for GEMM1
KH = HP // 128      # contraction tiles for GEMM2 (k=5 contracts 64 rows)
NP = 5              # full x1/x2 pair chunks; pair 5 is the 64|64 edge chunk
NCH = 2 * NP + 1    # 11 weight chunks of 128 columns
DSLICE = 512        # DIM slice width for GEMM2


def _token_slices(C):
    """Split C (mult of 128) into GEMM1 slice widths <=512, each >=256
    where possible (f32r runs 1 cyc/row only at N>=256)."""
    out = []
    rem = C
    while rem > 640:
        out.append(512)
        rem -= 512
    if rem > 512:
        a = (rem // 2 + 127) // 128 * 128
        out += [a, rem - a]
    elif rem:
        out.append(rem)
    return out
N_CORES = 8

MM_DT_NAME = os.environ.get("KERNEL_MM_DT", "f32r")  # f32 | f32r | bf16

_BUILD_CACHE = {}
LAST_RESULTS = None  # test harness reads exec_time_ns etc. from here


def _mm_view(ap, mm_dt):
    return ap


def _ensure_ntff_hook():
    """Profiling-only: register the ctypes NTFF hook (antenv.axon_hooks is
    not shipped in this container) and keep profile post-processing local."""
    import types

    import concourse.bass_utils as bu

    try:
        from antenv.axon_hooks import get_axon_ntff_profile_hook  # noqa: F401
    except ImportError:
        try:
            from trn_agent_boot.trn_boot import _ntff_profile_via_ctypes

            hook = _ntff_profile_via_ctypes("/opt/axon/libaxon_pjrt.so")
        except Exception:
            hook = None
        mod = types.ModuleType("antenv.axon_hooks")
        mod.get_axon_ntff_profile_hook = lambda: hook
        mod.set_axon_ntff_profile_hook = lambda h: None
        sys.modules["antenv.axon_hooks"] = mod
        import antenv

        antenv.axon_hooks = mod
    # keep artifacts local — no bucket in this container
    bu.upload_artifacts = lambda tmpdir: f"local://{tmpdir}"


def _install_drain_patch():
    """walrus 2026-05 rejects >1 sem wait on CTRL-class (Drain/NoOp) SP
    instructions; respell Tile's tail drain as a chain of 1-wait NOPs."""
    import concourse.mybir as mybir
    import concourse.tile as tile
    from concourse.tile import ScopedClock

    if getattr(tile.TileContext, "_drain_patch_installed", False):
        return

    def _patched(self, tick_clock, wait_clock):
        nc = self.nc
        nop_inst = nc.sync.nop(nofuse=True, hint="drain_waits")
        wait_clock.add_sem_waits(
            nop_inst.ins, ScopedClock({None: tick_clock.global_clock})
        )
        waits = list(nop_inst.ins.sync_info.on_wait or [])
        if len(waits) > 1:
            nop_inst.ins.sync_info.on_wait = waits[:1]
            for w in waits[1:]:
                extra = nc.sync.nop(nofuse=True, hint="drain_waits")
                extra.ins.sync_info = mybir.SyncInfo(on_wait=[w], on_update=[])
        nc.sync.drain()
        nc.all_engine_barrier()
        assert self.sems is not None
        popped = nc._tile_sem_poison_stack.pop()
        assert popped is self._sem_poison
        nc.clear_and_free_semaphores(list(self.sems.allocated().values()))
        nc.all_engine_barrier()

    tile.TileContext._drain_and_barrier = _patched
    tile.TileContext._drain_patch_installed = True


def _build_program(C, mm_dt, with_b12):
    """Build the single-core Bass program (SPMD: same program, per-core data)."""
    import concourse.bacc as bacc
    import concourse.bass as bass  # noqa: F401
    import concourse.mybir as mybir
    import concourse.tile as tile

    f32 = mybir.dt.float32
    if mm_dt == "bf16":
        io_dt = mybir.dt.bfloat16
    elif mm_dt == "f32r":
        io_dt = mybir.dt.float32r
    else:
        io_dt = f32

    SL = _token_slices(C)
    SOFF = [0]
    for w in SL:
        SOFF.append(SOFF[-1] + w)
    TN = len(SL)
    NT = C // 128        # token tiles for GEMM2

    nc = bacc.Bacc("TRN2", target_bir_lowering=False, debug=False,
                   enable_asserts=False, num_devices=N_CORES)

    # Host-packed partition-major layouts: every DMA below moves full
    # contiguous per-partition rows (128 descriptors, ~0.6us trigger each).
    xT = nc.dram_tensor("xT", [128, KD * C], io_dt, kind="ExternalInput")
    w12 = nc.dram_tensor("w12", [128, KD * NCH * 128], io_dt,
                         kind="ExternalInput")
    w3 = nc.dram_tensor("w3", [128, KH * DIM], io_dt, kind="ExternalInput")
    gt = nc.dram_tensor("gt", [128, NT], f32, kind="ExternalInput")
    if with_b12:
        b12 = nc.dram_tensor("b12", [128, NCH], f32, kind="ExternalInput")
    out = nc.dram_tensor("out", [C, DIM], f32, kind="ExternalOutput")

    silu = mybir.ActivationFunctionType.Silu

    with tile.TileContext(nc) as tc:
        with (
            tc.tile_pool(name="weights", bufs=1) as wpool,
            tc.tile_pool(name="tmp", bufs=3) as tpool,
            tc.tile_pool(name="ps_g1", bufs=2, space="PSUM") as pspool1,
            tc.tile_pool(name="ps_g2", bufs=4, space="PSUM") as pspool2,
        ):
            w12sb = wpool.tile([128, KD * NCH * 128], io_dt, tag="w12sb")
            xTsb = wpool.tile([128, KD * C], io_dt, tag="xTsb")
            w3sb = wpool.tile([128, KH, DIM], io_dt, tag="w3sb")
            gsb = wpool.tile([128, NT], f32, tag="gsb")
            hid = wpool.tile([128, KH, C], io_dt, tag="hid")
            if with_b12:
                b12sb = wpool.tile([128, NCH], f32, tag="b12sb")
                nc.sync.dma_start(b12sb[:], b12[:])

            # Input DMAs, ordered by first consumption. The first pair/slice
            # are split in half so the PE ramps a few us earlier.
            PW = KD * 256      # w12 cols per full pair block (pair 5: KD*128)

            def _w12_dma(p, k0, k1):
                pw2 = 256 if p < NP else 128
                nc.sync.dma_start(
                    w12sb[:, p * PW + k0 * pw2:p * PW + k1 * pw2],
                    w12[:, p * PW + k0 * pw2:p * PW + k1 * pw2])

            def _xT_dma(n, k0, k1):
                w = SL[n]
                base = SOFF[n] * KD
                nc.sync.dma_start(
                    xTsb[:, base + k0 * w:base + k1 * w],
                    xT[:, base + k0 * w:base + k1 * w])

            _w12_dma(0, 0, KD // 2)
            _xT_dma(0, 0, KD // 2)
            _w12_dma(0, KD // 2, KD)
            _xT_dma(0, KD // 2, KD)
            nc.sync.dma_start(gsb[:], gt[:])
            _w12_dma(1, 0, KD)
            _w12_dma(2, 0, KD)
            nc.sync.dma_start(w3sb[:, :KH // 2, :], w3[:, :KH // 2 * DIM])
            _w12_dma(3, 0, KD)
            _w12_dma(4, 0, KD)
            _w12_dma(5, 0, KD)  # pair-5 edge block, KD*128 wide
            nc.sync.dma_start(w3sb[:, KH // 2:, :], w3[:, KH // 2 * DIM:])
            for n in range(1, TN):
                _xT_dma(n, 0, KD)

            def _gemm2_tile(t):
                tsl = slice(t * 128, (t + 1) * 128)
                o = tpool.tile([128, DIM], f32, tag="o")
                for d in range(DIM // DSLICE):
                    pso = pspool2.tile([128, DSLICE], f32, tag="pso")
                    for k in range(KH):
                        kp = 128 if k < KH - 1 else 64
                        nc.tensor.matmul(
                            pso,
                            _mm_view(hid[:kp, k, tsl], mm_dt),
                            _mm_view(w3sb[:kp, k, d * DSLICE:(d + 1) * DSLICE], mm_dt),
                            start=(k == 0), stop=(k == KH - 1),
                        )
                    nc.vector.tensor_scalar_mul(
                        o[:, d * DSLICE:(d + 1) * DSLICE], pso, gsb[:, t:t + 1])
                nc.sync.dma_start(out[tsl, :], o)

            t_emitted = 0
            for n in range(TN):
                w = SL[n]
                ns = slice(SOFF[n], SOFF[n] + w)
                xbase = SOFF[n] * KD
                # GEMM1 + SwiGLU: 5 full pair chunks + the 64|64 edge chunk
                for p in range(NP):
                    ps1 = pspool1.tile([128, 512], f32, tag="ps1", name="ps1")[:, :w]
                    ps2 = pspool1.tile([128, 512], f32, tag="ps2", name="ps2")[:, :w]
                    for k in range(KD):
                        nc.tensor.matmul(
                            ps1,
                            _mm_view(w12sb[:, p * PW + k * 256:p * PW + k * 256 + 128], mm_dt),
                            _mm_view(xTsb[:, xbase + k * w:xbase + (k + 1) * w], mm_dt),
                            start=(k == 0), stop=(k == KD - 1),
                        )
                    for k in range(KD):
                        nc.tensor.matmul(
                            ps2,
                            _mm_view(w12sb[:, p * PW + k * 256 + 128:p * PW + (k + 1) * 256], mm_dt),
                            _mm_view(xTsb[:, xbase + k * w:xbase + (k + 1) * w], mm_dt),
                            start=(k == 0), stop=(k == KD - 1),
                        )
                    s = tpool.tile([128, 512], f32, tag="s", name="s")[:, :w]
                    if with_b12:
                        nc.scalar.activation(s, ps1, silu,
                                             bias=b12sb[:, 2 * p:2 * p + 1])
                        nc.vector.tensor_scalar_add(ps2, ps2,
                                                    b12sb[:, 2 * p + 1:2 * p + 2])
                    else:
                        nc.scalar.activation(s, ps1, silu)
                    nc.vector.tensor_mul(out=hid[:, p, ns], in0=s, in1=ps2)

                # edge chunk: x1 rows 0:64, x2 rows 64:128 of one psum tile
                ps5 = pspool1.tile([128, 512], f32, tag="ps1", name="ps5")[:, :w]
                for k in range(KD):
                    nc.tensor.matmul(
                        ps5,
                        _mm_view(w12sb[:, NP * PW + k * 128:NP * PW + (k + 1) * 128], mm_dt),
                        _mm_view(xTsb[:, xbase + k * w:xbase + (k + 1) * w], mm_dt),
                        start=(k == 0), stop=(k == KD - 1),
                    )
                s5 = tpool.tile([128, 512], f32, tag="s", name="s5")[:64, :w]
                if with_b12:
                    nc.scalar.activation(s5, ps5[:64], silu,
                                         bias=b12sb[:64, NCH - 1:NCH])
                    nc.vector.tensor_scalar_add(ps5[64:], ps5[64:],
                                                b12sb[64:, NCH - 1:NCH])
                else:
                    nc.scalar.activation(s5, ps5[:64], silu)
                nc.vector.tensor_mul(out=hid[:64, NP, ns], in0=s5, in1=ps5[64:])

                # GEMM2 lags one slice behind GEMM1 so the PE never waits
                # on the SwiGLU chain at a slice seam.
                for t in range(t_emitted, SOFF[n] // 128):
                    _gemm2_tile(t)
                t_emitted = SOFF[n] // 128
            for t in range(t_emitted, NT):
                _gemm2_tile(t)

    nc.compile()
    return nc


def _np_io_dtype(mm_dt):
    if mm_dt == "bf16":
        import ml_dtypes

        return np.dtype(ml_dtypes.bfloat16)
    return np.dtype(np.float32)


def kernel(hidden_states, top_k_weights, W12, b12, W3, b3, top_k_index):
    global LAST_RESULTS
    from concourse.bass_utils import run_bass_kernel_spmd

    hs = np.asarray(hidden_states, dtype=np.float32)
    wts = np.asarray(top_k_weights, dtype=np.float32)
    idx = np.asarray(top_k_index)
    W12n = np.asarray(W12, dtype=np.float32)
    b12n = np.asarray(b12, dtype=np.float32)
    W3n = np.asarray(W3, dtype=np.float32)
    b3n = np.asarray(b3, dtype=np.float32)

    T = hs.shape[0]
    mm_dt = MM_DT_NAME
    io_np = _np_io_dtype(mm_dt)

    # ---- routing on host ----
    gates = np.zeros((E, T), np.float32)
    for k in range(TOPK):
        np.add.at(gates, (idx[:, k], np.arange(T)), wts[:, k])
    tok = [np.nonzero((idx == e).any(axis=1))[0] for e in range(E)]
    maxlen = max(256, max(len(t) for t in tok))
    C = ((maxlen + 127) // 128) * 128
    NT = C // 128

    with_b12 = bool(np.any(b12n))
    key = (C, mm_dt, with_b12)
    if key not in _BUILD_CACHE:
        _BUILD_CACHE[key] = _build_program(C, mm_dt, with_b12)
    nc = _BUILD_CACHE[key]

    # ---- per-core inputs ----
    in_maps = []
    for e in range(E):
        te = tok[e]
        ne = len(te)
        X = np.zeros((C, DIM), np.float32)
        X[:ne] = hs[te]
        # per-slice [128, KD, w] partition-major packs, concatenated
        blocks = []
        off = 0
        for w in _token_slices(C):
            blk = X[off:off + w].reshape(w, KD, 128).transpose(2, 1, 0)
            blocks.append(np.ascontiguousarray(blk).reshape(128, -1))
            off += w
        xT = np.concatenate(blocks, axis=1).astype(io_np, copy=False)

        # 11 chunks of 128 cols: 5 pairs [x1|x2] + edge [x1last64|x2last64]
        w12c = np.zeros((DIM, NCH, 128), np.float32)
        b12c = np.zeros((NCH, 128), np.float32)
        for p in range(NP):
            w12c[:, 2 * p, :] = W12n[e][:, p * 128:(p + 1) * 128]
            w12c[:, 2 * p + 1, :] = W12n[e][:, H + p * 128:H + (p + 1) * 128]
            b12c[2 * p] = b12n[e][p * 128:(p + 1) * 128]
            b12c[2 * p + 1] = b12n[e][H + p * 128:H + (p + 1) * 128]
        w12c[:, NCH - 1, :64] = W12n[e][:, NP * 128:H]
        w12c[:, NCH - 1, 64:] = W12n[e][:, H + NP * 128:2 * H]
        b12c[NCH - 1, :64] = b12n[e][NP * 128:H]
        b12c[NCH - 1, 64:] = b12n[e][H + NP * 128:2 * H]
        # pack pair-major: pairs p<5 -> [KD, 256] blocks, edge -> [KD, 128]
        pair_blocks = []
        for p in range(NP):
            blk = w12c[:, 2 * p:2 * p + 2, :].reshape(KD, 128, 256)
            pair_blocks.append(
                np.ascontiguousarray(blk.transpose(1, 0, 2)).reshape(128, -1))
        blk = w12c[:, NCH - 1, :].reshape(KD, 128, 128)
        pair_blocks.append(
            np.ascontiguousarray(blk.transpose(1, 0, 2)).reshape(128, -1))
        w12p = np.concatenate(pair_blocks, axis=1)
        b12p = b12c  # [NCH, 128]

        w3p = np.zeros((HP, DIM), np.float32)
        w3p[:H] = W3n[e]
        w3p = np.ascontiguousarray(
            w3p.reshape(KH, 128, DIM).transpose(1, 0, 2)).reshape(128, -1)

        g = np.zeros((C,), np.float32)
        g[:ne] = gates[e, te]
        gtile = np.ascontiguousarray(g.reshape(NT, 128).T)

        m = {
            "xT": xT,
            "w12": w12p.astype(io_np, copy=False),
            "w3": w3p.astype(io_np, copy=False),
            "gt": gtile,
        }
        if with_b12:
            m["b12"] = np.ascontiguousarray(b12p.T)
        in_maps.append(m)

    trace = bool(os.environ.get("KERNEL_TRACE"))
    kw = {}
    if trace:
        _ensure_ntff_hook()
        kw = {"trace_cores": list(range(N_CORES)), "stitch_traces": False}
    res = run_bass_kernel_spmd(nc, in_maps, list(range(N_CORES)), trace=trace, **kw)
    LAST_RESULTS = res

    # ---- combine on host ----
    out = np.zeros((T, DIM), np.float32)
    for e in range(E):
        te = tok[e]
        out[te] += res.results[e]["out"][:len(te)]
    if np.any(b3n):
        out += gates.T @ b3n
    return out


# revision 14
# speedup vs baseline: 1.0683x; 1.0683x over previous
"""MoE (DeepSeek-style naive top-k routing + per-expert SwiGLU) on 8 Trainium2 cores.

Strategy: expert parallelism with host-side token dispatch/combine.
  - Host computes the routing (top_k_index/top_k_weights -> per-expert token
    lists + combine gates), gathers each expert's tokens into a padded
    capacity-C buffer, and hands core e exactly expert e's weights + tokens.
  - Each core runs dense SwiGLU over its C tokens:
        Y^T = W12p^T @ X^T          (GEMM1, contraction over DIM=1024)
        hidden^T = silu(x1) * x2    (pair-interleaved chunk layout)
        out = (hidden^T)^T @ W3     (GEMM2, contraction over H, padded to 768)
    with the per-token combine gate folded into the PSUM->SBUF copy of the
    GEMM2 result.
  - Host scatter-adds the 8 per-expert partial outputs into the [T, DIM] out.

W12 columns are permuted on host into 6 chunks of [x1_p (128) | x2_p (128)]
so that silu(x1)*x2 pairs are partition-aligned in the Y^T layout. H=704 is
zero-padded to 768 (x1 pad cols give silu(0)*0 = 0, harmless for GEMM2).
"""

import os
import sys

for _p in ("/opt/trn_rl_repo",):
    if _p not in sys.path:
        sys.path.insert(0, _p)

import numpy as np

E = 8
DIM = 1024
H = 704
TOPK = 2
HP = 768            # H padded to a multiple of 128
KD = DIM // 128     # contraction tiles for GEMM1
KH = HP // 128      # contraction tiles for GEMM2
NP = HP // 128      # x1/x2 pair chunks (the last pair is zero-padded 64->128)
DSLICE = 512        # DIM slice width for GEMM2


def _token_slices(C):
    """Split C (mult of 128) into GEMM1 slice widths <=512, each >=256
    where possible (f32r runs 1 cyc/row only at N>=256)."""
    out = []
    rem = C
    while rem > 640:
        out.append(512)
        rem -= 512
    if rem > 512:
        a = (rem // 2 + 127) // 128 * 128
        out += [a, rem - a]
    elif rem:
        out.append(rem)
    return out
N_CORES = 8

MM_DT_NAME = os.environ.get("KERNEL_MM_DT", "f32r")  # f32 | f32r | bf16

_BUILD_CACHE = {}
LAST_RESULTS = None  # test harness reads exec_time_ns etc. from here


def _mm_view(ap, mm_dt):
    return ap


def _ensure_ntff_hook():
    """Profiling-only: register the ctypes NTFF hook (antenv.axon_hooks is
    not shipped in this container) and keep profile post-processing local."""
    import types

    import concourse.bass_utils as bu

    try:
        from antenv.axon_hooks import get_axon_ntff_profile_hook  # noqa: F401
    except ImportError:
        try:
            from trn_agent_boot.trn_boot import _ntff_profile_via_ctypes

            hook = _ntff_profile_via_ctypes("/opt/axon/libaxon_pjrt.so")
        except Exception:
            hook = None
        mod = types.ModuleType("antenv.axon_hooks")
        mod.get_axon_ntff_profile_hook = lambda: hook
        mod.set_axon_ntff_profile_hook = lambda h: None
        sys.modules["antenv.axon_hooks"] = mod
        import antenv

        antenv.axon_hooks = mod
    # keep artifacts local — no bucket in this container
    bu.upload_artifacts = lambda tmpdir: f"local://{tmpdir}"


def _install_drain_patch():
    """walrus 2026-05 rejects >1 sem wait on CTRL-class (Drain/NoOp) SP
    instructions; respell Tile's tail drain as a chain of 1-wait NOPs."""
    import concourse.mybir as mybir
    import concourse.tile as tile
    from concourse.tile import ScopedClock

    if getattr(tile.TileContext, "_drain_patch_installed", False):
        return

    def _patched(self, tick_clock, wait_clock):
        nc = self.nc
        nop_inst = nc.sync.nop(nofuse=True, hint="drain_waits")
        wait_clock.add_sem_waits(
            nop_inst.ins, ScopedClock({None: tick_clock.global_clock})
        )
        waits = list(nop_inst.ins.sync_info.on_wait or [])
        if len(waits) > 1:
            nop_inst.ins.sync_info.on_wait = waits[:1]
            for w in waits[1:]:
                extra = nc.sync.nop(nofuse=True, hint="drain_waits")
                extra.ins.sync_info = mybir.SyncInfo(on_wait=[w], on_update=[])
        nc.sync.drain()
        nc.all_engine_barrier()
        assert self.sems is not None
        popped = nc._tile_sem_poison_stack.pop()
        assert popped is self._sem_poison
        nc.clear_and_free_semaphores(list(self.sems.allocated().values()))
        nc.all_engine_barrier()

    tile.TileContext._drain_and_barrier = _patched
    tile.TileContext._drain_patch_installed = True


def _build_program(C, mm_dt, with_b12):
    """Build the single-core Bass program (SPMD: same program, per-core data)."""
    import concourse.bacc as bacc
    import concourse.bass as bass  # noqa: F401
    import concourse.mybir as mybir
    import concourse.tile as tile

    f32 = mybir.dt.float32
    if mm_dt == "bf16":
        io_dt = mybir.dt.bfloat16
    elif mm_dt == "f32r":
        io_dt = mybir.dt.float32r
    else:
        io_dt = f32

    SL = _token_slices(C)
    SOFF = [0]
    for w in SL:
        SOFF.append(SOFF[-1] + w)
    TN = len(SL)
    NT = C // 128        # token tiles for GEMM2

    nc = bacc.Bacc("TRN2", target_bir_lowering=False, debug=False,
                   enable_asserts=False, num_devices=N_CORES)

    # Host-packed partition-major layouts: every DMA below moves full
    # contiguous per-partition rows (128 descriptors, ~0.6us trigger each).
    xT = nc.dram_tensor("xT", [128, KD * C], io_dt, kind="ExternalInput")
    w12 = nc.dram_tensor("w12", [128, NP * KD * 256], io_dt,
                         kind="ExternalInput")
    w3 = nc.dram_tensor("w3", [128, KH * DIM], io_dt, kind="ExternalInput")
    gt = nc.dram_tensor("gt", [128, NT], f32, kind="ExternalInput")
    if with_b12:
        b12 = nc.dram_tensor("b12", [128, 2 * NP], f32, kind="ExternalInput")
    out = nc.dram_tensor("out", [C, DIM], f32, kind="ExternalOutput")

    silu = mybir.ActivationFunctionType.Silu

    with tile.TileContext(nc) as tc:
        with (
            tc.tile_pool(name="weights", bufs=1) as wpool,
            tc.tile_pool(name="tmp", bufs=3) as tpool,
            tc.tile_pool(name="ps_g1", bufs=2, space="PSUM") as pspool1,
            tc.tile_pool(name="ps_g2", bufs=4, space="PSUM") as pspool2,
        ):
            w12sb = wpool.tile([128, NP, KD, 256], io_dt, tag="w12sb")
            xTsb = wpool.tile([128, KD * C], io_dt, tag="xTsb")
            w3sb = wpool.tile([128, KH, DIM], io_dt, tag="w3sb")
            gsb = wpool.tile([128, NT], f32, tag="gsb")
            hid = wpool.tile([128, KH, C], io_dt, tag="hid")
            if with_b12:
                b12sb = wpool.tile([128, 2 * NP], f32, tag="b12sb")
                nc.sync.dma_start(b12sb[:], b12[:])

            # Input DMAs, ordered by first consumption. The first pair/slice
            # are split in half so the PE ramps a few us earlier.
            PW = KD * 256      # w12 cols per pair block

            def _w12_dma(p, k0, k1):
                nc.sync.dma_start(
                    w12sb[:, p, k0:k1, :],
                    w12[:, p * PW + k0 * 256:p * PW + k1 * 256])

            def _xT_dma(n, k0, k1):
                w = SL[n]
                base = SOFF[n] * KD
                nc.sync.dma_start(
                    xTsb[:, base + k0 * w:base + k1 * w],
                    xT[:, base + k0 * w:base + k1 * w])

            _w12_dma(0, 0, KD // 2)
            _xT_dma(0, 0, KD // 2)
            _w12_dma(0, KD // 2, KD)
            _xT_dma(0, KD // 2, KD)
            nc.sync.dma_start(gsb[:], gt[:])
            _w12_dma(1, 0, KD)
            _w12_dma(2, 0, KD)
            nc.sync.dma_start(w3sb[:, :KH // 2, :], w3[:, :KH // 2 * DIM])
            _w12_dma(3, 0, KD)
            _w12_dma(4, 0, KD)
            _w12_dma(5, 0, KD)
            nc.sync.dma_start(w3sb[:, KH // 2:, :], w3[:, KH // 2 * DIM:])
            for n in range(1, TN):
                _xT_dma(n, 0, KD)

            def _gemm2_tile(t):
                tsl = slice(t * 128, (t + 1) * 128)
                o = tpool.tile([128, DIM], f32, tag="o")
                for d in range(DIM // DSLICE):
                    pso = pspool2.tile([128, DSLICE], f32, tag="pso")
                    for k in range(KH):
                        nc.tensor.matmul(
                            pso,
                            _mm_view(hid[:, k, tsl], mm_dt),
                            _mm_view(w3sb[:, k, d * DSLICE:(d + 1) * DSLICE], mm_dt),
                            start=(k == 0), stop=(k == KH - 1),
                        )
                    nc.vector.tensor_scalar_mul(
                        o[:, d * DSLICE:(d + 1) * DSLICE], pso, gsb[:, t:t + 1])
                nc.sync.dma_start(out[tsl, :], o)

            t_emitted = 0
            for n in range(TN):
                w = SL[n]
                ns = slice(SOFF[n], SOFF[n] + w)
                xbase = SOFF[n] * KD
                # GEMM1 + SwiGLU for this token slice, all 6 pair chunks
                for p in range(NP):
                    ps1 = pspool1.tile([128, 512], f32, tag="ps1", name="ps1")[:, :w]
                    ps2 = pspool1.tile([128, 512], f32, tag="ps2", name="ps2")[:, :w]
                    for k in range(KD):
                        nc.tensor.matmul(
                            ps1,
                            _mm_view(w12sb[:, p, k, 0:128], mm_dt),
                            _mm_view(xTsb[:, xbase + k * w:xbase + (k + 1) * w], mm_dt),
                            start=(k == 0), stop=(k == KD - 1),
                        )
                    for k in range(KD):
                        nc.tensor.matmul(
                            ps2,
                            _mm_view(w12sb[:, p, k, 128:256], mm_dt),
                            _mm_view(xTsb[:, xbase + k * w:xbase + (k + 1) * w], mm_dt),
                            start=(k == 0), stop=(k == KD - 1),
                        )
                    s = tpool.tile([128, 512], f32, tag="s", name="s")[:, :w]
                    if with_b12:
                        nc.scalar.activation(s, ps1, silu,
                                             bias=b12sb[:, 2 * p:2 * p + 1])
                        nc.vector.tensor_scalar_add(ps2, ps2,
                                                    b12sb[:, 2 * p + 1:2 * p + 2])
                    else:
                        nc.scalar.activation(s, ps1, silu)
                    nc.vector.tensor_mul(out=hid[:, p, ns], in0=s, in1=ps2)

                # GEMM2 lags one slice behind GEMM1 so the PE never waits
                # on the SwiGLU chain at a slice seam.
                for t in range(t_emitted, SOFF[n] // 128):
                    _gemm2_tile(t)
                t_emitted = SOFF[n] // 128
            for t in range(t_emitted, NT):
                _gemm2_tile(t)

    nc.compile()
    return nc


def _np_io_dtype(mm_dt):
    if mm_dt == "bf16":
        import ml_dtypes

        return np.dtype(ml_dtypes.bfloat16)
    return np.dtype(np.float32)


def kernel(hidden_states, top_k_weights, W12, b12, W3, b3, top_k_index):
    global LAST_RESULTS
    from concourse.bass_utils import run_bass_kernel_spmd

    hs = np.asarray(hidden_states, dtype=np.float32)
    wts = np.asarray(top_k_weights, dtype=np.float32)
    idx = np.asarray(top_k_index)
    W12n = np.asarray(W12, dtype=np.float32)
    b12n = np.asarray(b12, dtype=np.float32)
    W3n = np.asarray(W3, dtype=np.float32)
    b3n = np.asarray(b3, dtype=np.float32)

    T = hs.shape[0]
    mm_dt = MM_DT_NAME
    io_np = _np_io_dtype(mm_dt)

    # ---- routing on host ----
    gates = np.zeros((E, T), np.float32)
    for k in range(TOPK):
        np.add.at(gates, (idx[:, k], np.arange(T)), wts[:, k])
    tok = [np.nonzero((idx == e).any(axis=1))[0] for e in range(E)]
    maxlen = max(256, max(len(t) for t in tok))
    C = ((maxlen + 127) // 128) * 128
    NT = C // 128

    with_b12 = bool(np.any(b12n))
    key = (C, mm_dt, with_b12)
    if key not in _BUILD_CACHE:
        _BUILD_CACHE[key] = _build_program(C, mm_dt, with_b12)
    nc = _BUILD_CACHE[key]

    # ---- per-core inputs ----
    in_maps = []
    for e in range(E):
        te = tok[e]
        ne = len(te)
        X = np.zeros((C, DIM), np.float32)
        X[:ne] = hs[te]
        # per-slice [128, KD, w] partition-major packs, concatenated
        blocks = []
        off = 0
        for w in _token_slices(C):
            blk = X[off:off + w].reshape(w, KD, 128).transpose(2, 1, 0)
            blocks.append(np.ascontiguousarray(blk).reshape(128, -1))
            off += w
        xT = np.concatenate(blocks, axis=1).astype(io_np, copy=False)

        w12p = np.zeros((DIM, NP, 256), np.float32)
        b12p = np.zeros((2 * HP,), np.float32)
        for p in range(NP):
            c0, c1 = p * 128, min(H, (p + 1) * 128)
            wd = c1 - c0
            if wd <= 0:
                continue
            w12p[:, p, :wd] = W12n[e][:, c0:c1]
            w12p[:, p, 128:128 + wd] = W12n[e][:, H + c0:H + c1]
            b12p[p * 256:p * 256 + wd] = b12n[e][c0:c1]
            b12p[p * 256 + 128:p * 256 + 128 + wd] = b12n[e][H + c0:H + c1]
        # [128, NP, KD, 256] partition-major pack
        w12p = np.ascontiguousarray(
            w12p.reshape(KD, 128, NP, 256).transpose(1, 2, 0, 3)
        ).reshape(128, -1)

        w3p = np.zeros((HP, DIM), np.float32)
        w3p[:H] = W3n[e]
        w3p = np.ascontiguousarray(
            w3p.reshape(KH, 128, DIM).transpose(1, 0, 2)).reshape(128, -1)

        g = np.zeros((C,), np.float32)
        g[:ne] = gates[e, te]
        gtile = np.ascontiguousarray(g.reshape(NT, 128).T)

        m = {
            "xT": xT,
            "w12": w12p.astype(io_np, copy=False),
            "w3": w3p.astype(io_np, copy=False),
            "gt": gtile,
        }
        if with_b12:
            m["b12"] = np.ascontiguousarray(b12p.reshape(2 * NP, 128).T)
        in_maps.append(m)

    trace = bool(os.environ.get("KERNEL_TRACE"))
    kw = {}
    if trace:
        _ensure_ntff_hook()
        kw = {"trace_cores": list(range(N_CORES)), "stitch_traces": False}
    res = run_bass_kernel_spmd(nc, in_maps, list(range(N_CORES)), trace=trace, **kw)
    LAST_RESULTS = res

    # ---- combine on host ----
    out = np.zeros((T, DIM), np.float32)
    for e in range(E):
        te = tok[e]
        out[te] += res.results[e]["out"][:len(te)]
    if np.any(b3n):
        out += gates.T @ b3n
    return out
